# revision 1
# baseline (speedup 1.0000x reference)
"""Self-contained Trainium2 kernel for the dense transformer block problem.

kernel(**inputs) takes the FULL inputs (as produced by the reference
setup_inputs), shards them across 8 NeuronCores (2 cores per batch element,
causal-balanced parity split of query blocks), runs a Bass/Tile SPMD kernel,
and reassembles the full [B, T, C] output.
"""
"""Transformer block (pre-LN attention + MLP) for trn2, 8-core SPMD.

Sharding: 2 cores per batch element (B=4). Within a pair, query blocks of 128
tokens are split by parity (core parity p owns global blocks {2j+p}), which
balances causal attention work. Each core computes K/V for the full sequence
of its batch element (redundant within the pair) so there are no collectives.

Per-core layouts:
  activations for matmuls flow transposed: [C_chunk x 128 partitions, tokens], f32r
  attention: scoresT [keys, q] (f32r matmul) -> +mask bias -> exp (ACT) -> weiT bf16
  V is bf16, augmented with a ones column; AV accumulates [attn^T ; rowsum] in PSUM
  softmax normalization via PE transpose + per-partition reciprocal
"""
import sys
sys.path.insert(0, '/opt/trn_rl_repo')
import numpy as np
from contextlib import ExitStack

import concourse.bacc as bacc
import concourse.tile as tile
import concourse.mybir as mybir
from concourse.masks import make_identity

F32 = mybir.dt.float32
F32R = mybir.dt.float32r
BF16 = mybir.dt.bfloat16
AF = mybir.ActivationFunctionType
ALU = mybir.AluOpType

B, T, C, H, DH = 4, 2048, 1024, 16, 64
N_CORES = 8
TOK = 1024          # own tokens per core
NB = TOK // 128     # 8 own query blocks
KB = T // 128       # 16 key blocks
CCH = C // 128      # 8 channel chunks
FF = 4 * C          # 4096
FCH = FF // 128     # 32 ff chunks
EPS = 1e-5
NEG = -1e30

IN_NAMES = ["xfull", "xown", "qpos", "Wq", "Wk", "Wv", "Wp", "bp",
            "W1", "b1", "W2", "b2", "qbias", "kbias", "vbias"]


def _score_chunks(nq):
    """split nq (multiple of 128) into pieces, avoiding <256 pieces when possible"""
    out = []
    rem = nq
    while rem > 0:
        if rem == 640:
            take = 384
        elif rem >= 512:
            take = 512
        else:
            take = rem
        out.append(take)
        rem -= take
    return out


def build(nc, reps=1):
    """Trace the SPMD program into nc (a bacc.Bacc). Call nc.compile() after.

    Weight inputs arrive pre-folded on the host:
      Wq/Wk/Wv = diag(g1) @ W (dtype f32r);  qbias/kbias/vbias = be1 @ W
      W1 = diag(g2) @ W1 (f32r);  b1 = b1 + be2 @ W1
      Wp, W2 plain f32r.  g/be tensors are consumed host-side only.
    """
    def din(name, shape, dt=F32):
        return nc.dram_tensor(name, shape, dt, kind="ExternalInput")

    xfull_d = din("xfull", [T, C])
    xown_d = din("xown", [TOK, C])
    qpos_d = din("qpos", [NB, 128])
    Wq_d = din("Wq", [C, C], F32R); Wk_d = din("Wk", [C, C], F32R)
    Wv_d = din("Wv", [C, C], F32R); Wp_d = din("Wp", [C, C], F32R)
    bp_d = din("bp", [1, C]); W1_d = din("W1", [C, FF], F32R); b1_d = din("b1", [1, FF])
    W2_d = din("W2", [FF, C], F32R); b2_d = din("b2", [1, C])
    qb_d = din("qbias", [NB, 128])   # be1 @ Wq, laid out [pair, dh-stacked 128]
    kb_d = din("kbias", [NB, 128])   # be1 @ Wk
    vb_d = din("vbias", [1, C])      # be1 @ Wv
    out_d = nc.dram_tensor("out", [TOK, C], F32, kind="ExternalOutput")
    x2_d = nc.dram_tensor("x2_scratch", [TOK, C], F32)  # internal DRAM scratch
    attnT_d = nc.dram_tensor("attnT_scratch", [C, TOK], F32R)  # [dh-stacked C, own tokens]

    Wqv = Wq_d.ap().rearrange("(o p) m -> o p m", p=128)
    Wkv = Wk_d.ap().rearrange("(o p) m -> o p m", p=128)
    Wvv = Wv_d.ap().rearrange("(o p) m -> o p m", p=128)
    Wpv = Wp_d.ap().rearrange("(o p) m -> o p m", p=128)
    W1v = W1_d.ap().rearrange("(o p) m -> o p m", p=128)
    W2v = W2_d.ap().rearrange("(o p) m -> p o m", p=128)  # [128, 32, 1024]
    xf = xfull_d.ap()
    xo = xown_d.ap()

    for _rep in range(reps):
        _build_one(nc, locals())
    return IN_NAMES


def _build_one(nc, env):
    (xfull_d, xown_d, qpos_d, Wq_d, Wk_d, Wv_d, Wp_d, bp_d, W1_d, b1_d, W2_d,
     b2_d, qb_d, kb_d, vb_d, out_d, x2_d, attnT_d, Wqv, Wkv, Wvv, Wpv, W1v, W2v,
     xf, xo) = (
        env[k] for k in ["xfull_d", "xown_d", "qpos_d", "Wq_d", "Wk_d", "Wv_d",
                         "Wp_d", "bp_d", "W1_d", "b1_d", "W2_d", "b2_d", "qb_d",
                         "kb_d", "vb_d", "out_d", "x2_d", "attnT_d", "Wqv", "Wkv",
                         "Wvv", "Wpv", "W1v", "W2v", "xf", "xo"])
    import concourse.tile as tile
    from contextlib import ExitStack
    with tile.TileContext(nc) as tc, ExitStack() as top:
        const = top.enter_context(tc.tile_pool(name="const", bufs=1))
        ident = const.tile([128, 128], F32)
        make_identity(nc, ident[:])
        eps_t = const.tile([128, 1], F32)
        nc.vector.memset(eps_t[:], EPS)

        def ln_stats(nc, pool, x_ap):
            n = x_ap.shape[-1] // 512
            xg = x_ap.rearrange("p (n f) -> p n f", f=512)
            stats = pool.tile([128, n, 6], F32, tag="ln_stats")
            mv = pool.tile([128, 2], F32, tag="ln_mv")
            for i in range(n):
                nc.vector.bn_stats(stats[:, i], xg[:, i])
            nc.vector.bn_aggr(mv[:], stats[:])
            rstd = pool.tile([128, 1], F32, tag="ln_rstd")
            nc.scalar.activation(rstd[:], mv[:, 1:2], AF.Sqrt, bias=eps_t[:])
            nc.vector.reciprocal(rstd[:], rstd[:])
            return mv[:, 0:1], rstd

        def ln_apply(nc, pool, out_ap, x_ap, mean, rstd):
            # out = (x - mu) * rstd on ACT: Identity(x * rstd + (-mu * rstd))
            nmr = pool.tile([128, 1], F32, tag="ln_nmr")
            nc.vector.tensor_scalar(nmr[:], mean, rstd[:], -1.0,
                                    op0=ALU.mult, op1=ALU.mult)
            nc.scalar.activation(out_ap, x_ap, AF.Identity,
                                 bias=nmr[:], scale=rstd[:])

        # ============ Stage A: LN1 over full T -> hT [128, CCH, T] f32r ============
        es_h = ExitStack()
        hp = es_h.enter_context(tc.tile_pool(name="hT", bufs=1, side="right"))
        hT = hp.tile([128, CCH, T], F32R)
        with tc.tile_pool(name="stA", bufs=3) as stA, \
             tc.tile_pool(name="stA_ps", bufs=3, space="PSUM") as psA:
            for tb in range(T // 128):
                x_t = stA.tile([128, C], F32, tag="x_t")
                nc.sync.dma_start(x_t[:], xf[tb * 128:(tb + 1) * 128, :])
                mean, rstd = ln_stats(nc, stA, x_t[:])
                hrow = stA.tile([128, C], F32, tag="hrow")
                ln_apply(nc, stA, hrow[:], x_t[:], mean, rstd)
                for cc in range(CCH):
                    pt = psA.tile([128, 128], F32, tag="psA_t")
                    nc.tensor.transpose(pt[:], hrow[:, cc * 128:(cc + 1) * 128], ident[:])
                    eng = nc.scalar.copy if cc % 2 == 0 else nc.vector.tensor_copy
                    eng(hT[:, cc, tb * 128:(tb + 1) * 128], pt[:])

        # ============ Stage B1: V (token-major, bf16, ones-augmented) ============
        es_qkv = ExitStack()
        vp = es_qkv.enter_context(tc.tile_pool(name="Vp", bufs=1))
        V_sb = vp.tile([128, KB, H, 65], F32R)
        ones_f = vp.tile([128, 1], F32)
        nc.vector.memset(ones_f[:], 1.0)
        ones_r = vp.tile([128, 1], F32R)
        nc.vector.tensor_copy(ones_r[:], ones_f[:])
        nc.vector.tensor_copy(V_sb[:, :, :, 64:65],
                              ones_r[:, 0:1, None, None].to_broadcast([128, KB, H, 1]))
        with tc.tile_pool(name="stB1a", bufs=2) as stB1a, \
             tc.tile_pool(name="stB1c", bufs=1) as stB1c, \
             tc.tile_pool(name="stB1_ps", bufs=2, space="PSUM") as psB1:
            vb_b = stB1c.tile([128, C], F32)
            nc.sync.dma_start(vb_b[:], vb_d.ap().to_broadcast([128, C]))
            for grp in range(2):
                wv_g = stB1a.tile([128, CCH, 512], F32R, tag="wv_g")
                nc.sync.dma_start(wv_g[:], Wvv.transpose([1, 0, 2])[:, :, grp * 512:(grp + 1) * 512])
                for tb in range(KB):
                    pv = psB1.tile([128, 512], F32, tag="pv")
                    for cc in range(CCH):
                        nc.tensor.matmul(pv[:], hT[:, cc, tb * 128:(tb + 1) * 128],
                                         wv_g[:, cc], start=(cc == 0), stop=(cc == CCH - 1))
                    nc.vector.tensor_tensor(
                        V_sb[:, tb, grp * 8:(grp + 1) * 8, 0:64],
                        pv[:].rearrange("p (h d) -> p h d", d=64),
                        vb_b[:, grp * 512:(grp + 1) * 512].rearrange("p (h d) -> p h d", d=64),
                        ALU.add)

        # ============ Stage B2: KT [128(dh pair-stacked), pair, T] f32r ============
        ktp = es_qkv.enter_context(tc.tile_pool(name="KTp", bufs=1))
        KT = ktp.tile([128, CCH, T], F32R)
        with tc.tile_pool(name="stB2", bufs=2) as stB2, \
             tc.tile_pool(name="stB2c", bufs=1) as stB2c, \
             tc.tile_pool(name="stB2_ps", bufs=3, space="PSUM") as psB2:
            kb_sb = stB2c.tile([128, NB], F32)
            nc.sync.dma_start(kb_sb[:], kb_d.ap().rearrange("o p -> p o"))
            for pair in range(CCH):
                wk_p = stB2.tile([128, CCH, 128], F32R, tag="wk_p")
                nc.sync.dma_start(wk_p[:], Wkv.transpose([1, 0, 2])[:, :, pair * 128:(pair + 1) * 128])
                for nt in range(T // 512):
                    pk = psB2.tile([128, 512], F32, tag="pk")
                    for cc in range(CCH):
                        nc.tensor.matmul(pk[:], wk_p[:, cc],
                                         hT[:, cc, nt * 512:(nt + 1) * 512],
                                         start=(cc == 0), stop=(cc == CCH - 1))
                    nc.vector.tensor_scalar(KT[:, pair, nt * 512:(nt + 1) * 512], pk[:],
                                            kb_sb[:, pair:pair + 1], None, op0=ALU.add)

        # ============ Stage A': LN1 of own rows -> hTown; then B3: QT ============
        es_h.close()  # free hT
        es_ho = ExitStack()
        hop = es_ho.enter_context(tc.tile_pool(name="hTown", bufs=1, side="right"))
        hTown = hop.tile([128, CCH, TOK], F32R)
        with tc.tile_pool(name="stA2", bufs=3) as stA2, \
             tc.tile_pool(name="stA2_ps", bufs=3, space="PSUM") as psA2:
            for tb in range(NB):
                x_t = stA2.tile([128, C], F32, tag="x_t2")
                nc.sync.dma_start(x_t[:], xo[tb * 128:(tb + 1) * 128, :])
                mean, rstd = ln_stats(nc, stA2, x_t[:])
                hrow = stA2.tile([128, C], F32, tag="hrow2")
                ln_apply(nc, stA2, hrow[:], x_t[:], mean, rstd)
                for cc in range(CCH):
                    pt = psA2.tile([128, 128], F32, tag="psA2_t")
                    nc.tensor.transpose(pt[:], hrow[:, cc * 128:(cc + 1) * 128], ident[:])
                    eng = nc.scalar.copy if cc % 2 == 0 else nc.vector.tensor_copy
                    eng(hTown[:, cc, tb * 128:(tb + 1) * 128], pt[:])

        qtp = es_qkv.enter_context(tc.tile_pool(name="QTp", bufs=1))
        QT = qtp.tile([128, CCH, TOK], F32R)
        with tc.tile_pool(name="stB3", bufs=2) as stB3, \
             tc.tile_pool(name="stB3c", bufs=1) as stB3c, \
             tc.tile_pool(name="stB3_ps", bufs=3, space="PSUM") as psB3:
            qb_sb = stB3c.tile([128, NB], F32)
            nc.sync.dma_start(qb_sb[:], qb_d.ap().rearrange("o p -> p o"))
            for pair in range(CCH):
                wq_p = stB3.tile([128, CCH, 128], F32R, tag="wq_p")
                nc.sync.dma_start(wq_p[:], Wqv.transpose([1, 0, 2])[:, :, pair * 128:(pair + 1) * 128])
                for nt in range(TOK // 512):
                    pq = psB3.tile([128, 512], F32, tag="pq")
                    for cc in range(CCH):
                        nc.tensor.matmul(pq[:], wq_p[:, cc],
                                         hTown[:, cc, nt * 512:(nt + 1) * 512],
                                         start=(cc == 0), stop=(cc == CCH - 1))
                    nc.vector.tensor_scalar(QT[:, pair, nt * 512:(nt + 1) * 512], pq[:],
                                            qb_sb[:, pair:pair + 1], None, op0=ALU.add)
        es_ho.close()  # free hTown

        # ---------- mask constants (scoped to attention) ----------
        es_mask = ExitStack()
        maskp = es_mask.enter_context(tc.tile_pool(name="maskp", bufs=1, side="right"))
        kp_i = maskp.tile([128, KB], mybir.dt.int32)
        nc.gpsimd.iota(kp_i[:], pattern=[[128, KB]], base=0, channel_multiplier=1)
        kp_f = maskp.tile([128, KB], F32)
        nc.vector.tensor_copy(kp_f[:], kp_i[:])
        qb = maskp.tile([128, NB, 128], F32)
        for j in range(NB):
            nc.sync.dma_start(qb[:, j], qpos_d.ap()[j:j + 1, :].to_broadcast([128, 128]))
        biasm = maskp.tile([128, NB, 2, 128], F32)
        for j in range(NB):
            for t in range(2):
                # m01[p_key, f_q] = (qpos_j[f] >= keypos(k=2j+t)[p])
                nc.vector.tensor_scalar(
                    biasm[:, j, t], qb[:, j], kp_f[:, 2 * j + t:2 * j + t + 1], None,
                    op0=ALU.is_ge)

        # ============ Stage C: attention ============
        with tc.tile_pool(name="stC", bufs=3) as stC, \
             tc.tile_pool(name="stC_att_ps", bufs=2, space="PSUM") as psCa, \
             tc.tile_pool(name="stC_s_ps", bufs=2, space="PSUM") as psCs, \
             tc.tile_pool(name="stC_t_ps", bufs=2, space="PSUM") as psCt:
            for h in range(H):
                pair, off = h // 2, 64 * (h % 2)
                ps_att = psCa.tile([128, TOK], F32, tag="ps_att")
                for k in range(KB):
                    jmin = k // 2
                    q0 = jmin * 128
                    nq = TOK - q0
                    weiT = stC.tile([128, TOK], F32R, tag="weiT")
                    qa = 0
                    while qa < nq:  # one 1-bank psum tile + one exp per 512 cols
                        qn = min(512, nq - qa)
                        ps_s = psCs.tile([128, 512], F32, tag="ps_s")
                        nc.tensor.matmul(
                            ps_s[:, 0:qn],
                            KT[off:off + 64, pair, k * 128:(k + 1) * 128],
                            QT[off:off + 64, pair, q0 + qa:q0 + qa + qn],
                            start=True, stop=True)
                        nc.scalar.activation(weiT[:, qa:qa + qn], ps_s[:, 0:qn],
                                             AF.Exp, scale=0.125)
                        qa += qn
                    nc.vector.tensor_tensor(weiT[:, 0:128], weiT[:, 0:128],
                                            biasm[:, jmin, k - 2 * jmin], ALU.mult)
                    # AV: one matmul per 512-col PSUM bank (start=True must
                    # clear a whole bank, so groups are bank-aligned)
                    if k <= 7:  # bank 0: q cols [q0, 512)
                        nc.tensor.matmul(
                            ps_att[0:65, q0:512],
                            V_sb[:, k, h, :],
                            weiT[:, 0:512 - q0],
                            start=(k == 0), stop=(k == 7))
                    b1lo = max(512, q0)  # bank 1: q cols [b1lo, 1024)
                    nc.tensor.matmul(
                        ps_att[0:65, b1lo:TOK],
                        V_sb[:, k, h, :],
                        weiT[:, b1lo - q0:TOK - q0],
                        start=(k == 0), stop=(k == KB - 1))
                # normalize + transpose back into attnT
                for j in range(NB):
                    sb_at = stC.tile([128, 128], F32, tag="sb_at")
                    nc.vector.tensor_copy(sb_at[0:65, :], ps_att[0:65, j * 128:(j + 1) * 128])
                    pt1 = psCt.tile([128, 128], F32, tag="ptn")
                    nc.tensor.transpose(pt1[:], sb_at[:], ident[:])
                    recip = stC.tile([128, 1], F32, tag="recip")
                    nc.vector.reciprocal(recip[:], pt1[:, 64:65])
                    attn_j = stC.tile([128, 64], F32, tag="attn_j")
                    nc.vector.tensor_scalar_mul(attn_j[:], pt1[:, 0:64], recip[:])
                    pt2 = psCt.tile([128, 128], F32, tag="ptn")
                    nc.tensor.transpose(pt2[0:64, :], attn_j[:], ident[:])
                    att_st = stC.tile([64, 128], F32R, tag="att_st")
                    nc.vector.tensor_copy(att_st[:], pt2[0:64, :])
                    nc.sync.dma_start(
                        attnT_d.ap()[pair * 128 + off:pair * 128 + off + 64,
                                     j * 128:(j + 1) * 128], att_st[:])
        es_qkv.close()   # free V, KT, QT
        attnTv = attnT_d.ap().rearrange("(o p) t -> o p t", p=128)

        # ============ Stage D: Wp proj + residual + LN2 ============
        es_x2 = ExitStack()
        x2p = es_x2.enter_context(tc.tile_pool(name="x2h2", bufs=1))
        h2T = x2p.tile([128, CCH, TOK], F32R)
        with tc.tile_pool(name="stD", bufs=2) as stD, \
             tc.tile_pool(name="stD_c", bufs=1) as stDc, \
             tc.tile_pool(name="stD_ps", bufs=2, space="PSUM") as psD, \
             tc.tile_pool(name="stD_t_ps", bufs=2, space="PSUM") as psDt:
            bpb = stDc.tile([128, C], F32)
            nc.sync.dma_start(bpb[:], bp_d.ap().to_broadcast([128, C]))
            for nt in range(TOK // 512):
                pT_sb = stD.tile([128, CCH, 512], F32, tag="pT_sb")
                at_nt = stD.tile([128, CCH, 512], F32R, tag="at_nt")
                nc.sync.dma_start(at_nt[:],
                                  attnTv.transpose([1, 0, 2])[:, :, nt * 512:(nt + 1) * 512])
                for co in range(CCH):
                    pp = psD.tile([128, 512], F32, tag="pp")
                    wp_c = stD.tile([128, CCH, 128], F32R, tag="wp_c")
                    nc.sync.dma_start(wp_c[:], Wpv.transpose([1, 0, 2])[:, :, co * 128:(co + 1) * 128])
                    for cc in range(CCH):
                        nc.tensor.matmul(pp[:], wp_c[:, cc],
                                         at_nt[:, cc],
                                         start=(cc == 0), stop=(cc == CCH - 1))
                    nc.scalar.copy(pT_sb[:, co], pp[:])
                for sub in range(4):
                    tb = nt * 4 + sub
                    x2_t = stD.tile([128, C], F32, tag="x2_t")
                    xo_t = stD.tile([128, C], F32, tag="xo_t")
                    nc.sync.dma_start(xo_t[:], xo[tb * 128:(tb + 1) * 128, :])
                    for co in range(CCH):
                        ptd = psDt.tile([128, 128], F32, tag="ptd")
                        nc.tensor.transpose(ptd[:], pT_sb[:, co, sub * 128:(sub + 1) * 128],
                                            ident[:])
                        nc.vector.tensor_tensor(x2_t[:, co * 128:(co + 1) * 128], ptd[:],
                                                xo_t[:, co * 128:(co + 1) * 128], ALU.add)
                    nc.vector.tensor_tensor(x2_t[:], x2_t[:], bpb[:], ALU.add)
                    nc.sync.dma_start(x2_d.ap()[tb * 128:(tb + 1) * 128, :], x2_t[:])
                    # LN2
                    mean, rstd = ln_stats(nc, stD, x2_t[:])
                    h2row = stD.tile([128, C], F32, tag="h2row")
                    ln_apply(nc, stD, h2row[:], x2_t[:], mean, rstd)
                    for cc in range(CCH):
                        pt = psDt.tile([128, 128], F32, tag="ptd2")
                        nc.tensor.transpose(pt[:], h2row[:, cc * 128:(cc + 1) * 128], ident[:])
                        eng = nc.scalar.copy if cc % 2 == 0 else nc.vector.tensor_copy
                        eng(h2T[:, cc, tb * 128:(tb + 1) * 128], pt[:])
        es_mask.close()  # free mask constants

        # ============ Stage E: MLP split by ff-halves (W1/W2 streamed once) ====
        # ff2_sb accumulates the two ff-half partial products in SBUF.
        es_ff2 = ExitStack()
        ff2p = es_ff2.enter_context(tc.tile_pool(name="ff2sb", bufs=1))
        ff2_sb = ff2p.tile([128, CCH, TOK], F32)
        with tc.tile_pool(name="stF_c", bufs=1) as stFc:
            b1p = stFc.tile([128, FCH], F32)
            nc.sync.dma_start(b1p[:], b1_d.ap().rearrange("x (o p) -> p (x o)", p=128))
            b2b = stFc.tile([128, C], F32)
            nc.sync.dma_start(b2b[:], b2_d.ap().to_broadcast([128, C]))
            FH = FCH // 2  # 16 ff chunks per half
            for fh in range(2):
                es_half = ExitStack()
                ffp = es_half.enter_context(tc.tile_pool(name="ff1T", bufs=1))
                ff1T = ffp.tile([128, FH, TOK], F32R)
                with tc.tile_pool(name="stE1", bufs=2) as stE1, \
                     tc.tile_pool(name="stE1_ps", bufs=2, space="PSUM") as psE1:
                    for fog in range(4):
                        w1g = stE1.tile([128, CCH, 512], F32R, tag="w1g")
                        nc.sync.dma_start(
                            w1g[:], W1v.transpose([1, 0, 2])
                            [:, :, fh * 2048 + fog * 512:fh * 2048 + (fog + 1) * 512])
                        for f4 in range(4):
                            fo = fog * 4 + f4          # local ff chunk in this half
                            for nt in range(TOK // 512):
                                pf = psE1.tile([128, 512], F32, tag="pf")
                                for cc in range(CCH):
                                    nc.tensor.matmul(
                                        pf[:], w1g[:, cc, f4 * 128:(f4 + 1) * 128],
                                        h2T[:, cc, nt * 512:(nt + 1) * 512],
                                        start=(cc == 0), stop=(cc == CCH - 1))
                                nc.scalar.activation(
                                    ff1T[:, fo, nt * 512:(nt + 1) * 512], pf[:], AF.Relu,
                                    bias=b1p[:, fh * FH + fo:fh * FH + fo + 1])
                with tc.tile_pool(name="stE2", bufs=2) as stE2, \
                     tc.tile_pool(name="stE2_ps", bufs=2, space="PSUM") as psE2:
                    for co in range(CCH):
                        w2c = stE2.tile([128, FH, 128], F32R, tag="w2c")
                        nc.sync.dma_start(
                            w2c[:], W2v[:, fh * FH:(fh + 1) * FH, co * 128:(co + 1) * 128])
                        for nt in range(TOK // 512):
                            p2 = psE2.tile([128, 512], F32, tag="p2")
                            for fo in range(FH):
                                nc.tensor.matmul(p2[:], w2c[:, fo],
                                                 ff1T[:, fo, nt * 512:(nt + 1) * 512],
                                                 start=(fo == 0), stop=(fo == FH - 1))
                            dst = ff2_sb[:, co, nt * 512:(nt + 1) * 512]
                            if fh == 0:
                                nc.scalar.copy(dst, p2[:])
                            else:
                                nc.vector.tensor_tensor(dst, dst, p2[:], ALU.add)
                es_half.close()
            # ============ Stage F: transpose + residual + output ============
            with tc.tile_pool(name="stF", bufs=2) as stF, \
                 tc.tile_pool(name="stF_ps", bufs=2, space="PSUM") as psF:
                for tb in range(NB):
                    out_t = stF.tile([128, C], F32, tag="out_t")
                    x2_t = stF.tile([128, C], F32, tag="x2r_t")
                    nc.sync.dma_start(x2_t[:], x2_d.ap()[tb * 128:(tb + 1) * 128, :])
                    sub = tb % 4
                    for co in range(CCH):
                        ptf = psF.tile([128, 128], F32, tag="ptf")
                        nc.tensor.transpose(
                            ptf[:], ff2_sb[:, co, tb * 128:(tb + 1) * 128], ident[:])
                        nc.vector.tensor_tensor(out_t[:, co * 128:(co + 1) * 128], ptf[:],
                                                x2_t[:, co * 128:(co + 1) * 128], ALU.add)
                    nc.vector.tensor_tensor(out_t[:], out_t[:], b2b[:], ALU.add)
                    nc.sync.dma_start(out_d.ap()[tb * 128:(tb + 1) * 128, :], out_t[:])
        es_ff2.close()
        es_x2.close()


def make_nc():
    nc = bacc.Bacc("TRN2", target_bir_lowering=False, debug=False,
                   num_devices=N_CORES)
    build(nc)
    nc.compile()
    return nc


def shard_inputs(inputs):
    """Full inputs dict -> list of 8 per-core in_maps.

    Folds LN1 gain/bias into Wq/Wk/Wv (weights scaled by g1 per input channel,
    be1 contribution becomes an additive bias on q/k/v) and LN2's into W1/b1.
    """
    x = np.asarray(inputs["x"], np.float32)
    assert x.shape == (B, T, C)
    f64 = np.float64
    Wq = np.asarray(inputs["Wq"], f64); Wk = np.asarray(inputs["Wk"], f64)
    Wv = np.asarray(inputs["Wv"], f64); Wp = np.asarray(inputs["Wp"], np.float32)
    W1 = np.asarray(inputs["W1"], f64); W2 = np.asarray(inputs["W2"], np.float32)
    g1 = np.asarray(inputs["g1"], f64); be1 = np.asarray(inputs["be1"], f64)
    g2 = np.asarray(inputs["g2"], f64); be2 = np.asarray(inputs["be2"], f64)
    b1 = np.asarray(inputs["b1"], f64)
    shared = {
        "Wq": (g1[:, None] * Wq).astype(np.float32),
        "Wk": (g1[:, None] * Wk).astype(np.float32),
        "Wv": (g1[:, None] * Wv).astype(np.float32),
        "Wp": Wp, "W2": W2,
        "W1": (g2[:, None] * W1).astype(np.float32),
        "qbias": (be1 @ Wq).astype(np.float32).reshape(NB, 128),
        "kbias": (be1 @ Wk).astype(np.float32).reshape(NB, 128),
        "vbias": (be1 @ Wv).astype(np.float32).reshape(1, C),
        "b1": (b1 + be2 @ W1).astype(np.float32).reshape(1, FF),
        "bp": np.asarray(inputs["bp"], np.float32).reshape(1, C),
        "b2": np.asarray(inputs["b2"], np.float32).reshape(1, C),
    }
    in_maps = []
    for c in range(N_CORES):
        b, par = c // 2, c % 2
        gblocks = [2 * j + par for j in range(NB)]
        rows = np.concatenate([x[b, g * 128:(g + 1) * 128, :] for g in gblocks], 0)
        qpos = np.stack([np.arange(g * 128, (g + 1) * 128, dtype=np.float32)
                         for g in gblocks], 0)
        m = {"xfull": np.ascontiguousarray(x[b]),
             "xown": np.ascontiguousarray(rows), "qpos": qpos}
        m.update(shared)
        in_maps.append(m)
    return in_maps


def unshard_outputs(results):
    """list of per-core {'out': [TOK, C]} -> [B, T, C]"""
    out = np.zeros((B, T, C), np.float32)
    for c in range(N_CORES):
        b, par = c // 2, c % 2
        r = np.asarray(results[c]["out"])
        for j in range(NB):
            g = 2 * j + par
            out[b, g * 128:(g + 1) * 128, :] = r[j * 128:(j + 1) * 128, :]
    return out


_NC_CACHE = {}

def _get_nc():
    if "nc" not in _NC_CACHE:
        nc = bacc.Bacc("TRN2", target_bir_lowering=False, debug=False,
                       num_devices=N_CORES)
        build(nc, reps=1)
        nc.compile()
        _NC_CACHE["nc"] = nc
    return _NC_CACHE["nc"]


def kernel(**inputs):
    from concourse.bass_utils import run_bass_kernel_spmd
    nc = _get_nc()
    in_maps = shard_inputs(inputs)
    res = run_bass_kernel_spmd(nc, in_maps, core_ids=list(range(N_CORES)))
    return unshard_outputs(res.results)



# revision 2
# speedup vs baseline: 1.0310x; 1.0310x over previous
"""Self-contained Trainium2 kernel for the dense transformer block problem.

kernel(**inputs) takes FULL inputs, shards across 8 NeuronCores (2 cores per
batch element, causal-balanced parity split of query blocks), runs a Bass/Tile
SPMD kernel, and reassembles the full [B, T, C] output.

v2 design notes (vs v1 baseline):
- bf16 for all matmul operands (weights pre-cast on host; activations
  quantized at PSUM->SBUF copies). PE transposes run at 1 cyc/row.
- Wp and W2 matmuls run "swapped" (activations stationary, weights moving)
  so their outputs are token-major: kills the proj/ffn output transposes and
  keeps x2 / attnT / acc resident in SBUF (no DRAM roundtrips).
- Single PSUM pool for the whole program (tags: mm=1 bank x2, wide=2 banks
  x2, ptn=1 bank x2 -> exactly 8 banks) so phases overlap without pool
  alloc barriers.
- Biases folded: k/q biases via ACT Identity-bias on PSUM->SBUF copy; v bias
  added on the attnT copy (softmax weights sum to 1); bp and b2 injected via
  rank-1 ones-row matmul accumulation; b1 via ACT Relu bias.
- Causal mask multiply runs on the (otherwise idle) GPSIMD engine.
"""
import sys
sys.path.insert(0, '/opt/trn_rl_repo')
import numpy as np
from contextlib import ExitStack

import concourse.bacc as bacc
import concourse.tile as tile
import concourse.mybir as mybir
from concourse.masks import make_identity
from concourse.tile import add_dep_helper

F32 = mybir.dt.float32
BF16 = mybir.dt.bfloat16
I32 = mybir.dt.int32
AF = mybir.ActivationFunctionType
ALU = mybir.AluOpType

B, T, C, H, DH = 4, 2048, 1024, 16, 64
# Schraudolph exp for a few heads on DVE+Pool: exp(s*0.125) ~
# bitcast_f32(int32(SCH_A*s + SCH_B)); ~1.8% rms weight error, cancels in
# softmax normalization. Offloads ACT (the attention-phase bottleneck).
SCH_A = 0.125 * (1 << 23) / np.log(2.0)
SCH_B = float(127 * (1 << 23) - 486411)
N_CORES = 8
TOK = 1024          # own tokens per core
NB = TOK // 128     # 8 own query blocks
KB = T // 128       # 16 key blocks
CCH = C // 128      # 8 channel chunks
FF = 4 * C          # 4096
FCH = FF // 128     # 32 ff chunks
FH = FCH // 2       # 16 ff chunks per half
EPS = 1e-5

IN_NAMES = ["xfull", "xown", "qpos", "Wq", "Wk", "Wv", "Wp", "bp",
            "W1", "b1", "W2", "b2", "qbias", "kbias", "vbias"]


def build(nc, reps=1):
    """Trace the SPMD program into nc (a bacc.Bacc). Call nc.compile() after.

    Weight inputs arrive pre-folded on the host (bf16):
      Wq/Wk/Wv = diag(g1) @ W;  qbias/kbias/vbias = be1 @ W (f32)
      W1 = diag(g2) @ W1;  b1 = b1 + be2 @ W1 (f32)
      Wp, W2 plain.  bp/b2 in bf16 (rank-1 matmul injection).
    """
    def din(name, shape, dt=F32):
        return nc.dram_tensor(name, shape, dt, kind="ExternalInput")

    xfull_d = din("xfull", [T, C])
    xown_d = din("xown", [TOK, C])
    qpos_d = din("qpos", [NB, 128])
    Wq_d = din("Wq", [C, C], BF16); Wk_d = din("Wk", [C, C], BF16)
    Wv_d = din("Wv", [C, C], BF16); Wp_d = din("Wp", [C, C], BF16)
    bp_d = din("bp", [1, C], BF16)
    W1_d = din("W1", [C, FF], BF16); b1_d = din("b1", [1, FF])
    W2_d = din("W2", [FF, C], BF16); b2_d = din("b2", [1, C], BF16)
    qb_d = din("qbias", [NB, 128])   # be1 @ Wq, laid out [pair, dh-stacked 128]
    kb_d = din("kbias", [NB, 128])   # be1 @ Wk
    vb_d = din("vbias", [1, C])      # be1 @ Wv
    out_d = nc.dram_tensor("out", [TOK, C], F32, kind="ExternalOutput")

    env = dict(
        Wqv=Wq_d.ap().rearrange("(o p) m -> o p m", p=128),
        Wkv=Wk_d.ap().rearrange("(o p) m -> o p m", p=128),
        Wvv=Wv_d.ap().rearrange("(o p) m -> o p m", p=128),
        Wpv=Wp_d.ap().rearrange("(o p) m -> o p m", p=128),
        W1v=W1_d.ap().rearrange("(o p) m -> o p m", p=128),
        W2v=W2_d.ap().rearrange("(o p) m -> p o m", p=128),  # [128, 32, 1024]
        xf=xfull_d.ap(), xo=xown_d.ap(), qpos_d=qpos_d, bp_d=bp_d,
        b1_d=b1_d, b2_d=b2_d, qb_d=qb_d, kb_d=kb_d, vb_d=vb_d, out_d=out_d,
    )
    for _rep in range(reps):
        _build_one(nc, env)
    return IN_NAMES


def _build_one(nc, env):
    (Wqv, Wkv, Wvv, Wpv, W1v, W2v, xf, xo, qpos_d, bp_d, b1_d, b2_d, qb_d,
     kb_d, vb_d, out_d) = (
        env[k] for k in ["Wqv", "Wkv", "Wvv", "Wpv", "W1v", "W2v", "xf", "xo",
                         "qpos_d", "bp_d", "b1_d", "b2_d", "qb_d", "kb_d",
                         "vb_d", "out_d"])
    with tile.TileContext(nc) as tc, ExitStack() as top:
        # ---------------- constants ----------------
        const = top.enter_context(tc.tile_pool(name="const", bufs=1))
        ident_b = const.tile([128, 128], BF16)
        make_identity(nc, ident_b[:])
        eps_t = const.tile([128, 1], F32)
        nc.vector.memset(eps_t[:], EPS)
        ones1 = const.tile([1, 128], BF16)
        nc.vector.memset(ones1[:], 1.0)
        bp_sb = const.tile([1, C], BF16)
        nc.sync.dma_start(bp_sb[:], bp_d.ap())
        b2_sb = const.tile([1, C], BF16)
        nc.sync.dma_start(b2_sb[:], b2_d.ap())
        kb_sb = const.tile([128, NB], F32)
        nc.sync.dma_start(kb_sb[:], kb_d.ap().rearrange("o p -> p o"))
        qb_sb = const.tile([128, NB], F32)
        nc.sync.dma_start(qb_sb[:], qb_d.ap().rearrange("o p -> p o"))
        vb_sb = const.tile([128, CCH], F32)
        nc.sync.dma_start(vb_sb[:], vb_d.ap().rearrange("x (o p) -> p (x o)",
                                                        p=128))
        b1p = const.tile([128, FCH], F32)
        nc.sync.dma_start(b1p[:], b1_d.ap().rearrange("x (o p) -> p (x o)",
                                                      p=128))

        # mask constants (build tiles in a scoped pool; only biasm persists)
        biasm = const.tile([128, NB, 2, 128], BF16)
        with tc.tile_pool(name="maskb", bufs=1) as maskb:
            kp_i = maskb.tile([128, KB], mybir.dt.int32)
            nc.gpsimd.iota(kp_i[:], pattern=[[128, KB]], base=0,
                           channel_multiplier=1)
            kp_f = maskb.tile([128, KB], F32)
            nc.vector.tensor_copy(kp_f[:], kp_i[:])
            qb_pos = maskb.tile([128, NB, 128], F32)
            for j in range(NB):
                nc.sync.dma_start(
                    qb_pos[:, j],
                    qpos_d.ap()[j:j + 1, :].to_broadcast([128, 128]))
            for j in range(NB):
                for t in range(2):
                    # m[p_key, f_q] = (qpos_j[f] >= keypos(k=2j+t)[p])
                    nc.vector.tensor_scalar(
                        biasm[:, j, t], qb_pos[:, j],
                        kp_f[:, 2 * j + t:2 * j + t + 1], None, op0=ALU.is_ge)

        # PSUM pools (8 banks: mm 2 + pk 2 + wide 2 + ptn 2). pk is a
        # separate tag from mm so B2(pair+1)'s K matmuls don't queue behind
        # C(pair)'s score tiles in the mm slot FIFO.
        psMM = top.enter_context(tc.tile_pool(name="psMM", bufs=2,
                                              space="PSUM"))
        psK = top.enter_context(tc.tile_pool(name="psK", bufs=2,
                                             space="PSUM"))
        psW = top.enter_context(tc.tile_pool(name="psW", bufs=1, space="PSUM"))
        psT = top.enter_context(tc.tile_pool(name="psT", bufs=2, space="PSUM"))

        def ln_stats(nc, pool, x_ap):
            n = x_ap.shape[-1] // 512
            xg = x_ap.rearrange("p (n f) -> p n f", f=512)
            stats = pool.tile([128, n, 6], F32, tag="ln_stats")
            mv = pool.tile([128, 2], F32, tag="ln_mv")
            for i in range(n):
                nc.vector.bn_stats(stats[:, i], xg[:, i])
            nc.vector.bn_aggr(mv[:], stats[:])
            rstd = pool.tile([128, 1], F32, tag="ln_rstd")
            nc.scalar.activation(rstd[:], mv[:, 1:2], AF.Sqrt, bias=eps_t[:])
            nc.vector.reciprocal(rstd[:], rstd[:])
            return mv[:, 0:1], rstd

        def ln_apply(nc, pool, out_ap, x_ap, mean, rstd):
            # out = (x - mu) * rstd on ACT: Identity(x * rstd + (-mu * rstd))
            nmr = pool.tile([128, 1], F32, tag="ln_nmr")
            nc.vector.tensor_scalar(nmr[:], mean, rstd[:], -1.0,
                                    op0=ALU.mult, op1=ALU.mult)
            nc.scalar.activation(out_ap, x_ap, AF.Identity,
                                 bias=nmr[:], scale=rstd[:])

        # Pool allocation order is LIFO per side; allocate long-lived pools
        # first. hTown/hT/attnT cycle through the right side.
        # Emission order: A' -> A -> B3(Q) -> B1(V) -> [B2(K) + C] interleaved
        # per pair -> D -> E. B3/B1 fill PE while A'/A LN chains run; C's exp
        # (the ACT wall) starts as soon as pair 0's K is ready.
        es_h = ExitStack()
        hp = es_h.enter_context(tc.tile_pool(name="hT", bufs=1, side="right"))
        hT = hp.tile([128, CCH, T], BF16)
        es_ho = ExitStack()
        hop = es_ho.enter_context(tc.tile_pool(name="hTown", bufs=1,
                                               side="right"))
        hTown = hop.tile([128, CCH, TOK], BF16)
        es_qkv = ExitStack()
        vp = es_qkv.enter_context(tc.tile_pool(name="Vp", bufs=1))
        V_sb = vp.tile([128, KB, H, 65], BF16)
        ktp = es_qkv.enter_context(tc.tile_pool(name="KTp", bufs=1))
        KT = ktp.tile([128, CCH, T], BF16)
        qtp = es_qkv.enter_context(tc.tile_pool(name="QTp", bufs=1))
        QT = qtp.tile([128, CCH, TOK], BF16)
        wqkp = es_qkv.enter_context(tc.tile_pool(name="wqkp", bufs=1))
        wq_full = wqkp.tile([128, CCH, C], BF16, tag="wq")
        nc.gpsimd.dma_start(wq_full[:], Wqv.transpose([1, 0, 2]))
        wk_full = wqkp.tile([128, CCH, C], BF16, tag="wk")
        nc.gpsimd.dma_start(wk_full[:], Wkv.transpose([1, 0, 2]))
        es_B = ExitStack()
        stB = es_B.enter_context(tc.tile_pool(name="stB", bufs=2))
        wv_gs = []
        for grp in range(2):
            wv_g = stB.tile([128, CCH, 512], BF16, tag="wv_g")
            nc.gpsimd.dma_start(
                wv_g[:],
                Wvv.transpose([1, 0, 2])[:, :, grp * 512:(grp + 1) * 512])
            wv_gs.append(wv_g)
        ones_b = const.tile([128, 1], BF16)
        nc.vector.memset(ones_b[:], 1.0)
        nc.vector.tensor_copy(
            V_sb[:, :, :, 64:65],
            ones_b[:, 0:1, None, None].to_broadcast([128, KB, H, 1]))

        # ==== Stage A' (LN1 of own rows -> hTown) interleaved with B3 (QT):
        # each 512-token group of hTown feeds its Q matmuls immediately =====
        with tc.tile_pool(name="stA2", bufs=3) as stA2:
            for nt in range(2):
                for tb in range(4 * nt, 4 * nt + 4):
                    x_t = stA2.tile([128, C], F32, tag="x_t")
                    nc.sync.dma_start(x_t[:], xo[tb * 128:(tb + 1) * 128, :])
                    mean, rstd = ln_stats(nc, stA2, x_t[:])
                    hrow = stA2.tile([128, C], BF16, tag="hrow")
                    ln_apply(nc, stA2, hrow[:], x_t[:], mean, rstd)
                    for cg in range(2):
                        pt = psT.tile([128, 512], BF16, tag="ptn")
                        for i in range(4):
                            cc = cg * 4 + i
                            nc.tensor.matmul(
                                pt[:, i * 128:(i + 1) * 128],
                                hrow[:, cc * 128:(cc + 1) * 128], ident_b[:],
                                is_transpose=True,
                                start=(i == 0), stop=(i == 3))
                        eng = (nc.scalar.copy if cg == 0
                               else nc.vector.tensor_copy)
                        eng(hTown[:, cg * 4:(cg + 1) * 4,
                                  tb * 128:(tb + 1) * 128],
                            pt[:].rearrange("p (g f) -> p g f", f=128))
                for pair in range(CCH):
                    pq = psK.tile([128, 512], F32, tag="pk")
                    for cc in range(CCH):
                        nc.tensor.matmul(
                            pq[:], wq_full[:, cc, pair * 128:(pair + 1) * 128],
                            hTown[:, cc, nt * 512:(nt + 1) * 512],
                            start=(cc == 0), stop=(cc == CCH - 1))
                    nc.scalar.activation(QT[:, pair, nt * 512:(nt + 1) * 512],
                                         pq[:], AF.Identity,
                                         bias=qb_sb[:, pair:pair + 1])
        es_ho.close()   # free hTown

        # ============ Stage A: LN1 over full T -> hT [128, CCH, T] bf16 ====
        with tc.tile_pool(name="stA", bufs=3) as stA:
            for tb in range(T // 128):
                x_t = stA.tile([128, C], F32, tag="x_t")
                nc.scalar.dma_start(x_t[:], xf[tb * 128:(tb + 1) * 128, :])
                mean, rstd = ln_stats(nc, stA, x_t[:])
                hrow = stA.tile([128, C], BF16, tag="hrow")
                ln_apply(nc, stA, hrow[:], x_t[:], mean, rstd)
                for cg in range(2):
                    pt = psT.tile([128, 512], BF16, tag="ptn")
                    for i in range(4):
                        cc = cg * 4 + i
                        nc.tensor.matmul(
                            pt[:, i * 128:(i + 1) * 128],
                            hrow[:, cc * 128:(cc + 1) * 128], ident_b[:],
                            is_transpose=True, start=(i == 0), stop=(i == 3))
                    eng = (nc.scalar.copy if cg == 0
                           else nc.vector.tensor_copy)
                    eng(hT[:, cg * 4:(cg + 1) * 4, tb * 128:(tb + 1) * 128],
                        pt[:].rearrange("p (g f) -> p g f", f=128))

        # ============ Stage B1: V (token-major, bf16, ones-augmented) ======
        for grp in range(2):
            wv_g = wv_gs[grp]
            for tb in range(KB):
                pv = psMM.tile([128, 512], F32, tag="mm")
                for cc in range(CCH):
                    nc.tensor.matmul(pv[:], hT[:, cc, tb * 128:(tb + 1) * 128],
                                     wv_g[:, cc], start=(cc == 0),
                                     stop=(cc == CCH - 1))
                eng = nc.scalar.copy if tb % 2 == 0 else nc.vector.tensor_copy
                eng(V_sb[:, tb, grp * 8:(grp + 1) * 8, 0:64],
                    pv[:].rearrange("p (h d) -> p h d", d=64))
        es_B.close()  # free wv_g

        # ============ Stages B2 + C interleaved: K(pair) then attention ====
        es_at = ExitStack()
        atp = es_at.enter_context(tc.tile_pool(name="attnT", bufs=1,
                                               side="right"))
        attnT_sb = atp.tile([128, CCH, TOK], BF16)
        es_C = ExitStack()
        stC = es_C.enter_context(tc.tile_pool(name="stC", bufs=4))
        stC2 = es_C.enter_context(tc.tile_pool(name="stC2", bufs=2))
        av_gate = {}
        for pair in range(CCH):
            # ---- B2(pair): KT[:, pair, :] over full T ----
            # Gate B2(pair>=2) on attention progress of pair-2 so its matmuls
            # spread through the exp-paced attention tail instead of all
            # running up front.
            for nt in range(T // 512):
                pk = psK.tile([128, 512], F32, tag="pk")
                for cc in range(CCH):
                    mm = nc.tensor.matmul(
                        pk[:], wk_full[:, cc, pair * 128:(pair + 1) * 128],
                        hT[:, cc, nt * 512:(nt + 1) * 512],
                        start=(cc == 0), stop=(cc == CCH - 1))
                    if pair >= 2 and pair - 2 in av_gate:
                        add_dep_helper(mm.ins, av_gate[pair - 2].ins,
                                       sync=True, reason="spread B2 filler")
                # K bias on DVE (ACT is saturated by exp during this phase)
                nc.vector.tensor_scalar(KT[:, pair, nt * 512:(nt + 1) * 512],
                                        pk[:], kb_sb[:, pair:pair + 1], None,
                                        op0=ALU.add)
            # ---- C: the two heads of this pair ----
            for h in (2 * pair, 2 * pair + 1):
                off = 64 * (h % 2)
                ps_att = psW.tile([128, TOK], F32, tag="wide")
                for k in range(KB):
                    jmin = k // 2
                    q0 = jmin * 128
                    nq = TOK - q0
                    weiT = stC.tile([128, TOK], BF16, tag="weiT")
                    for qa in range(0, nq, 512):
                        qn = min(512, nq - qa)
                        if (k + qa // 512) % 2 == 0:
                            ps_s = psMM.tile([128, 512], F32, tag="mm")
                        else:
                            ps_s = psK.tile([128, 512], F32, tag="pk")
                        nc.tensor.matmul(
                            ps_s[:, 0:qn],
                            KT[off:off + 64, pair, k * 128:(k + 1) * 128],
                            QT[off:off + 64, pair, q0 + qa:q0 + qa + qn],
                            start=True, stop=True)
                        nc.scalar.activation(weiT[:, qa:qa + qn],
                                             ps_s[:, 0:qn], AF.Exp,
                                             scale=0.125)
                    nc.gpsimd.tensor_tensor(weiT[:, 0:128], weiT[:, 0:128],
                                            biasm[:, jmin, k - 2 * jmin],
                                            ALU.mult)
                    # AV: one matmul per 512-col PSUM bank
                    if k <= 7:  # bank 0: q cols [q0, 512)
                        nc.tensor.matmul(
                            ps_att[0:65, q0:512],
                            V_sb[:, k, h, :],
                            weiT[:, 0:512 - q0],
                            start=(k == 0), stop=(k == 7))
                    b1lo = max(512, q0)  # bank 1: q cols [b1lo, 1024)
                    av = nc.tensor.matmul(
                        ps_att[0:65, b1lo:TOK],
                        V_sb[:, k, h, :],
                        weiT[:, b1lo - q0:TOK - q0],
                        start=(k == 0), stop=(k == KB - 1))
                    if h % 2 == 1 and k == 8:
                        av_gate[pair] = av
                # normalize + transpose into attnT (SBUF). Row 64 of sb_at
                # gets the RECIPROCAL of the rowsum, so after the per-block
                # transpose the per-token 1/denom sits in column 64.
                sb_at = stC2.tile([128, TOK], BF16, tag="sb_at")
                # per-bank copies: bank 0 (cols 0:512) is final after k==7,
                # so it copies while bank 1 still accumulates
                nc.vector.tensor_copy(sb_at[0:65, 0:512],
                                      ps_att[0:65, 0:512])
                nc.vector.tensor_copy(sb_at[0:65, 512:TOK],
                                      ps_att[0:65, 512:TOK])
                for j in range(NB):
                    pt1 = psT.tile([128, 128], BF16, tag="ptn")
                    nc.tensor.transpose(pt1[:],
                                        sb_at[:, j * 128:(j + 1) * 128],
                                        ident_b[:])
                    recip = stC.tile([128, 1], F32, tag="recip")
                    nc.vector.reciprocal(recip[:], pt1[:, 64:65])
                    attn_j = stC.tile([128, 64], BF16, tag="attn_j")
                    nc.vector.tensor_scalar_mul(attn_j[:], pt1[:, 0:64],
                                                recip[:])
                    pt2 = psT.tile([128, 128], BF16, tag="ptn")
                    # transpose directly into partitions [off, off+64) —
                    # engines are lane-locked, so the copy below must be
                    # lane-aligned
                    nc.tensor.transpose(pt2[off:off + 64, :], attn_j[:],
                                        ident_b[:])
                    nc.vector.tensor_scalar(
                        attnT_sb[off:off + 64, pair, j * 128:(j + 1) * 128],
                        pt2[off:off + 64, :],
                        vb_sb[off:off + 64, pair:pair + 1],
                        None, op0=ALU.add)
        es_C.close()     # free weiT / sb_at working tiles
        es_qkv.close()   # free V, KT, QT

        # ============ Stage D: Wp proj (token-major) + residual + LN2 ======
        x2p = top.enter_context(tc.tile_pool(name="x2h2", bufs=1))
        x2_sb = x2p.tile([128, NB, C], F32)
        h2T = x2p.tile([128, CCH, TOK], BF16)
        wpp = ExitStack()
        wp_pool = wpp.enter_context(tc.tile_pool(name="wpf", bufs=1))
        wp_full = wp_pool.tile([128, CCH, C], BF16)
        nc.gpsimd.dma_start(wp_full[:], Wpv.transpose([1, 0, 2]))
        stD = wpp.enter_context(tc.tile_pool(name="stD", bufs=3))
        for tb in range(NB):
            psx0 = psMM.tile([128, 512], F32, tag="mm")
            psx1 = psK.tile([128, 512], F32, tag="pk")
            psx = [psx0, psx1]
            for cc in range(CCH):
                for ci, c0 in enumerate((0, 512)):
                    nc.tensor.matmul(
                        psx[ci][:],
                        attnT_sb[:, cc, tb * 128:(tb + 1) * 128],
                        wp_full[:, cc, c0:c0 + 512],
                        start=(cc == 0), stop=False)
            for ci, c0 in enumerate((0, 512)):  # + bp (rank-1 ones row)
                nc.tensor.matmul(psx[ci][:], ones1[:],
                                 bp_sb[:, c0:c0 + 512], start=False, stop=True)
            xo_t = stD.tile([128, C], F32, tag="xo_t")
            nc.sync.dma_start(xo_t[:], xo[tb * 128:(tb + 1) * 128, :])
            for ci, c0 in enumerate((0, 512)):
                nc.vector.tensor_tensor(x2_sb[:, tb, c0:c0 + 512], psx[ci][:],
                                        xo_t[:, c0:c0 + 512], ALU.add)
            # LN2
            mean, rstd = ln_stats(nc, stD, x2_sb[:, tb, :])
            h2row = stD.tile([128, C], BF16, tag="h2row")
            ln_apply(nc, stD, h2row[:], x2_sb[:, tb, :], mean, rstd)
            for cg in range(2):
                pt = psT.tile([128, 512], BF16, tag="ptn")
                for i in range(4):
                    cc = cg * 4 + i
                    nc.tensor.matmul(
                        pt[:, i * 128:(i + 1) * 128],
                        h2row[:, cc * 128:(cc + 1) * 128], ident_b[:],
                        is_transpose=True, start=(i == 0), stop=(i == 3))
                eng = nc.scalar.copy if cg == 0 else nc.vector.tensor_copy
                eng(h2T[:, cg * 4:(cg + 1) * 4, tb * 128:(tb + 1) * 128],
                    pt[:].rearrange("p (g f) -> p g f", f=128))
        wpp.close()    # free Wp + stage-D working tiles
        es_at.close()  # free attnT
        es_h.close()   # free hT (kept below attnT for LIFO pool order)

        # ============ Stage E: MLP (ff-halves; W2 swapped, token-major) ====
        accp = top.enter_context(tc.tile_pool(name="accp", bufs=1))
        acc_sb = accp.tile([128, NB, C], BF16)
        stE = top.enter_context(tc.tile_pool(name="stE", bufs=2))
        stE2 = top.enter_context(tc.tile_pool(name="stE2", bufs=1))
        stF = top.enter_context(tc.tile_pool(name="stF", bufs=3))
        for fh in range(2):
            es_half = ExitStack()
            ffp = es_half.enter_context(tc.tile_pool(name="ff1T", bufs=1))
            ff1T = ffp.tile([128, FH, TOK], BF16)
            for fog in range(4):
                w1g = stE.tile([128, CCH, 512], BF16, tag="w1g")
                nc.gpsimd.dma_start(
                    w1g[:], W1v.transpose([1, 0, 2])
                    [:, :, fh * 2048 + fog * 512:fh * 2048 + (fog + 1) * 512])
                for f4 in range(4):
                    fo = fog * 4 + f4          # local ff chunk in this half
                    for nt in range(TOK // 512):
                        pf = psMM.tile([128, 512], F32, tag="mm")
                        for cc in range(CCH):
                            nc.tensor.matmul(
                                pf[:], w1g[:, cc, f4 * 128:(f4 + 1) * 128],
                                h2T[:, cc, nt * 512:(nt + 1) * 512],
                                start=(cc == 0), stop=(cc == CCH - 1))
                        nc.scalar.activation(
                            ff1T[:, fo, nt * 512:(nt + 1) * 512], pf[:],
                            AF.Relu,
                            bias=b1p[:, fh * FH + fo:fh * FH + fo + 1])
            w2h = stE2.tile([128, FH, C], BF16, tag="w2h")
            nc.gpsimd.dma_start(w2h[:], W2v[:, fh * FH:(fh + 1) * FH, :])
            for tb in range(NB):
                p2a = psMM.tile([128, 512], F32, tag="mm")
                p2b = psK.tile([128, 512], F32, tag="pk")
                p2 = [p2a, p2b]
                for fo in range(FH):
                    for ci, c0 in enumerate((0, 512)):
                        nc.tensor.matmul(
                            p2[ci][:],
                            ff1T[:, fo, tb * 128:(tb + 1) * 128],
                            w2h[:, fo, c0:c0 + 512],
                            start=(fo == 0),
                            stop=(fh == 1 and fo == FH - 1))
                if fh == 0:
                    for ci, c0 in enumerate((0, 512)):  # + b2 (rank-1)
                        nc.tensor.matmul(p2[ci][:], ones1[:],
                                         b2_sb[:, c0:c0 + 512],
                                         start=False, stop=True)
                        eng = (nc.scalar.copy if ci == 0
                               else nc.vector.tensor_copy)
                        eng(acc_sb[:, tb, c0:c0 + 512], p2[ci][:])
                else:
                    out_t = stF.tile([128, C], F32, tag="out_t")
                    for ci, c0 in enumerate((0, 512)):
                        nc.vector.tensor_tensor(out_t[:, c0:c0 + 512],
                                                p2[ci][:],
                                                x2_sb[:, tb, c0:c0 + 512],
                                                ALU.add)
                    nc.vector.tensor_tensor(out_t[:], out_t[:],
                                            acc_sb[:, tb, :], ALU.add)
                    nc.sync.dma_start(out_d.ap()[tb * 128:(tb + 1) * 128, :],
                                      out_t[:])
            es_half.close()


def shard_inputs(inputs):
    """Full inputs dict -> list of 8 per-core in_maps.

    Folds LN1 gain/bias into Wq/Wk/Wv (weights scaled by g1 per input channel,
    be1 contribution becomes an additive bias on q/k/v) and LN2's into W1/b1.
    Weights are cast to bf16.
    """
    import ml_dtypes
    bf16 = ml_dtypes.bfloat16
    x = np.asarray(inputs["x"], np.float32)
    assert x.shape == (B, T, C)
    f64 = np.float64
    Wq = np.asarray(inputs["Wq"], f64); Wk = np.asarray(inputs["Wk"], f64)
    Wv = np.asarray(inputs["Wv"], f64); Wp = np.asarray(inputs["Wp"], f64)
    W1 = np.asarray(inputs["W1"], f64); W2 = np.asarray(inputs["W2"], f64)
    g1 = np.asarray(inputs["g1"], f64); be1 = np.asarray(inputs["be1"], f64)
    g2 = np.asarray(inputs["g2"], f64); be2 = np.asarray(inputs["be2"], f64)
    b1 = np.asarray(inputs["b1"], f64)
    shared = {
        "Wq": (g1[:, None] * Wq).astype(bf16),
        "Wk": (g1[:, None] * Wk).astype(bf16),
        "Wv": (g1[:, None] * Wv).astype(bf16),
        "Wp": Wp.astype(bf16), "W2": W2.astype(bf16),
        "W1": (g2[:, None] * W1).astype(bf16),
        "qbias": (be1 @ Wq).astype(np.float32).reshape(NB, 128),
        "kbias": (be1 @ Wk).astype(np.float32).reshape(NB, 128),
        "vbias": (be1 @ Wv).astype(np.float32).reshape(1, C),
        "b1": (b1 + be2 @ W1).astype(np.float32).reshape(1, FF),
        "bp": np.asarray(inputs["bp"], np.float32).astype(bf16).reshape(1, C),
        "b2": np.asarray(inputs["b2"], np.float32).astype(bf16).reshape(1, C),
    }
    in_maps = []
    for c in range(N_CORES):
        b, par = c // 2, c % 2
        gblocks = [2 * j + par for j in range(NB)]
        rows = np.concatenate([x[b, g * 128:(g + 1) * 128, :] for g in gblocks], 0)
        qpos = np.stack([np.arange(g * 128, (g + 1) * 128, dtype=np.float32)
                         for g in gblocks], 0)
        m = {"xfull": np.ascontiguousarray(x[b]),
             "xown": np.ascontiguousarray(rows), "qpos": qpos}
        m.update(shared)
        in_maps.append(m)
    return in_maps


def unshard_outputs(results):
    """list of per-core {'out': [TOK, C]} -> [B, T, C]"""
    out = np.zeros((B, T, C), np.float32)
    for c in range(N_CORES):
        b, par = c // 2, c % 2
        r = np.asarray(results[c]["out"])
        for j in range(NB):
            g = 2 * j + par
            out[b, g * 128:(g + 1) * 128, :] = r[j * 128:(j + 1) * 128, :]
    return out


_NC_CACHE = {}


def _get_nc():
    if "nc" not in _NC_CACHE:
        nc = bacc.Bacc("TRN2", target_bir_lowering=False, debug=False,
                       num_devices=N_CORES)
        build(nc, reps=1)
        nc.compile()
        _NC_CACHE["nc"] = nc
    return _NC_CACHE["nc"]


def kernel(**inputs):
    from concourse.bass_utils import run_bass_kernel_spmd
    nc = _get_nc()
    in_maps = shard_inputs(inputs)
    res = run_bass_kernel_spmd(nc, in_maps, core_ids=list(range(N_CORES)))
    return unshard_outputs(res.results)


# revision 3
# speedup vs baseline: 1.0829x; 1.0504x over previous
"""Self-contained Trainium2 kernel for the dense transformer block problem.

kernel(**inputs) takes FULL inputs, shards across 8 NeuronCores (2 cores per
batch element, causal-balanced parity split of query blocks), runs a Bass/Tile
SPMD kernel, and reassembles the full [B, T, C] output.

v2 design notes (vs v1 baseline):
- bf16 for all matmul operands (weights pre-cast on host; activations
  quantized at PSUM->SBUF copies). PE transposes run at 1 cyc/row.
- Wp and W2 matmuls run "swapped" (activations stationary, weights moving)
  so their outputs are token-major: kills the proj/ffn output transposes and
  keeps x2 / attnT / acc resident in SBUF (no DRAM roundtrips).
- Single PSUM pool for the whole program (tags: mm=1 bank x2, wide=2 banks
  x2, ptn=1 bank x2 -> exactly 8 banks) so phases overlap without pool
  alloc barriers.
- Biases folded: k/q biases via ACT Identity-bias on PSUM->SBUF copy; v bias
  added on the attnT copy (softmax weights sum to 1); bp and b2 injected via
  rank-1 ones-row matmul accumulation; b1 via ACT Relu bias.
- Causal mask multiply runs on the (otherwise idle) GPSIMD engine.
"""
import sys
sys.path.insert(0, '/opt/trn_rl_repo')
import numpy as np
from contextlib import ExitStack

import concourse.bacc as bacc
import concourse.tile as tile
import concourse.mybir as mybir
from concourse.masks import make_identity
from concourse.tile import add_dep_helper

F32 = mybir.dt.float32
BF16 = mybir.dt.bfloat16
I32 = mybir.dt.int32
AF = mybir.ActivationFunctionType
ALU = mybir.AluOpType

B, T, C, H, DH = 4, 2048, 1024, 16, 64
# Schraudolph exp for a few heads on DVE+Pool: exp(s*0.125) ~
# bitcast_f32(int32(SCH_A*s + SCH_B)); ~1.8% rms weight error, cancels in
# softmax normalization. Offloads ACT (the attention-phase bottleneck).
SCH_A = 0.125 * (1 << 23) / np.log(2.0)
SCH_B = float(127 * (1 << 23) - 486411)
N_CORES = 8
TOK = 1024          # own tokens per core
NB = TOK // 128     # 8 own query blocks
KB = T // 128       # 16 key blocks
CCH = C // 128      # 8 channel chunks
FF = 4 * C          # 4096
FCH = FF // 128     # 32 ff chunks
FH = FCH // 2       # 16 ff chunks per half
EPS = 1e-5

IN_NAMES = ["xfull", "xown", "qpos", "Wq", "Wk", "Wv", "Wp", "bp",
            "W1", "b1", "W2", "b2", "qbias", "kbias", "vbias"]


def build(nc, reps=1):
    """Trace the SPMD program into nc (a bacc.Bacc). Call nc.compile() after.

    Weight inputs arrive pre-folded on the host (bf16):
      Wq/Wk/Wv = diag(g1) @ W;  qbias/kbias/vbias = be1 @ W (f32)
      W1 = diag(g2) @ W1;  b1 = b1 + be2 @ W1 (f32)
      Wp, W2 plain.  bp/b2 in bf16 (rank-1 matmul injection).
    """
    def din(name, shape, dt=F32):
        return nc.dram_tensor(name, shape, dt, kind="ExternalInput")

    xfull_d = din("xfull", [T, C])
    xown_d = din("xown", [TOK, C])
    qpos_d = din("qpos", [NB, 128])
    Wq_d = din("Wq", [C, C], BF16); Wk_d = din("Wk", [C, C], BF16)
    Wv_d = din("Wv", [C, C], BF16); Wp_d = din("Wp", [C, C], BF16)
    bp_d = din("bp", [1, C], BF16)
    W1_d = din("W1", [C, FF], BF16); b1_d = din("b1", [1, FF])
    W2_d = din("W2", [FF, C], BF16); b2_d = din("b2", [1, C], BF16)
    qb_d = din("qbias", [NB, 128])   # be1 @ Wq, laid out [pair, dh-stacked 128]
    kb_d = din("kbias", [NB, 128])   # be1 @ Wk
    vb_d = din("vbias", [1, C])      # be1 @ Wv
    out_d = nc.dram_tensor("out", [TOK, C], F32, kind="ExternalOutput")

    env = dict(
        Wqv=Wq_d.ap().rearrange("(o p) m -> o p m", p=128),
        Wkv=Wk_d.ap().rearrange("(o p) m -> o p m", p=128),
        Wvv=Wv_d.ap().rearrange("(o p) m -> o p m", p=128),
        Wpv=Wp_d.ap().rearrange("(o p) m -> o p m", p=128),
        W1v=W1_d.ap().rearrange("(o p) m -> o p m", p=128),
        W2v=W2_d.ap().rearrange("(o p) m -> p o m", p=128),  # [128, 32, 1024]
        xf=xfull_d.ap(), xo=xown_d.ap(), qpos_d=qpos_d, bp_d=bp_d,
        b1_d=b1_d, b2_d=b2_d, qb_d=qb_d, kb_d=kb_d, vb_d=vb_d, out_d=out_d,
    )
    for _rep in range(reps):
        _build_one(nc, env)
    return IN_NAMES


def _build_one(nc, env):
    (Wqv, Wkv, Wvv, Wpv, W1v, W2v, xf, xo, qpos_d, bp_d, b1_d, b2_d, qb_d,
     kb_d, vb_d, out_d) = (
        env[k] for k in ["Wqv", "Wkv", "Wvv", "Wpv", "W1v", "W2v", "xf", "xo",
                         "qpos_d", "bp_d", "b1_d", "b2_d", "qb_d", "kb_d",
                         "vb_d", "out_d"])
    with tile.TileContext(nc) as tc, ExitStack() as top:
        # ---------------- constants ----------------
        const = top.enter_context(tc.tile_pool(name="const", bufs=1))
        ident_b = const.tile([128, 128], BF16)
        make_identity(nc, ident_b[:])
        eps_t = const.tile([128, 1], F32)
        nc.vector.memset(eps_t[:], EPS)
        ones1 = const.tile([1, 128], BF16)
        nc.vector.memset(ones1[:], 1.0)
        bp_sb = const.tile([1, C], BF16)
        nc.gpsimd.dma_start(bp_sb[:], bp_d.ap())
        b2_sb = const.tile([1, C], BF16)
        nc.gpsimd.dma_start(b2_sb[:], b2_d.ap())
        kb_sb = const.tile([128, NB], F32)
        nc.gpsimd.dma_start(kb_sb[:], kb_d.ap().rearrange("o p -> p o"))
        qb_sb = const.tile([128, NB], F32)
        nc.gpsimd.dma_start(qb_sb[:], qb_d.ap().rearrange("o p -> p o"))
        vb_sb = const.tile([128, CCH], F32)
        nc.gpsimd.dma_start(vb_sb[:], vb_d.ap().rearrange("x (o p) -> p (x o)",
                                                        p=128))
        b1p = const.tile([128, FCH], F32)
        nc.gpsimd.dma_start(b1p[:], b1_d.ap().rearrange("x (o p) -> p (x o)",
                                                      p=128))

        # mask constants (build tiles in a scoped pool; only biasm persists)
        biasm = const.tile([128, NB, 2, 128], BF16)
        with tc.tile_pool(name="maskb", bufs=1) as maskb:
            kp_i = maskb.tile([128, KB], mybir.dt.int32)
            nc.gpsimd.iota(kp_i[:], pattern=[[128, KB]], base=0,
                           channel_multiplier=1)
            kp_f = maskb.tile([128, KB], F32)
            nc.vector.tensor_copy(kp_f[:], kp_i[:])
            qb_pos = maskb.tile([128, NB, 128], F32)
            for j in range(NB):
                nc.gpsimd.dma_start(
                    qb_pos[:, j],
                    qpos_d.ap()[j:j + 1, :].to_broadcast([128, 128]))
            for j in range(NB):
                for t in range(2):
                    # m[p_key, f_q] = (qpos_j[f] >= keypos(k=2j+t)[p])
                    nc.vector.tensor_scalar(
                        biasm[:, j, t], qb_pos[:, j],
                        kp_f[:, 2 * j + t:2 * j + t + 1], None, op0=ALU.is_ge)

        # PSUM pools (8 banks: mm 2 + pk 2 + wide 2 + ptn 2). pk is a
        # separate tag from mm so B2(pair+1)'s K matmuls don't queue behind
        # C(pair)'s score tiles in the mm slot FIFO.
        psMM = top.enter_context(tc.tile_pool(name="psMM", bufs=2,
                                              space="PSUM"))
        psK = top.enter_context(tc.tile_pool(name="psK", bufs=2,
                                             space="PSUM"))
        psW = top.enter_context(tc.tile_pool(name="psW", bufs=1, space="PSUM"))
        psT = top.enter_context(tc.tile_pool(name="psT", bufs=2, space="PSUM"))

        def ln_stats(nc, pool, x_ap):
            n = x_ap.shape[-1] // 512
            xg = x_ap.rearrange("p (n f) -> p n f", f=512)
            stats = pool.tile([128, n, 6], F32, tag="ln_stats")
            mv = pool.tile([128, 2], F32, tag="ln_mv")
            for i in range(n):
                nc.vector.bn_stats(stats[:, i], xg[:, i])
            nc.vector.bn_aggr(mv[:], stats[:])
            rstd = pool.tile([128, 1], F32, tag="ln_rstd")
            nc.scalar.activation(rstd[:], mv[:, 1:2], AF.Sqrt, bias=eps_t[:])
            nc.vector.reciprocal(rstd[:], rstd[:])
            return mv[:, 0:1], rstd

        def ln_apply(nc, pool, out_ap, x_ap, mean, rstd):
            # out = (x - mu) * rstd on ACT: Identity(x * rstd + (-mu * rstd))
            nmr = pool.tile([128, 1], F32, tag="ln_nmr")
            nc.vector.tensor_scalar(nmr[:], mean, rstd[:], -1.0,
                                    op0=ALU.mult, op1=ALU.mult)
            nc.scalar.activation(out_ap, x_ap, AF.Identity,
                                 bias=nmr[:], scale=rstd[:])

        # Pool allocation order is LIFO per side; allocate long-lived pools
        # first. hTown/hT/attnT cycle through the right side.
        # Emission order: A' -> A -> B3(Q) -> B1(V) -> [B2(K) + C] interleaved
        # per pair -> D -> E. B3/B1 fill PE while A'/A LN chains run; C's exp
        # (the ACT wall) starts as soon as pair 0's K is ready.
        es_h = ExitStack()
        hp = es_h.enter_context(tc.tile_pool(name="hT", bufs=1, side="right"))
        hT = hp.tile([128, CCH, T], BF16)
        es_ho = ExitStack()
        hop = es_ho.enter_context(tc.tile_pool(name="hTown", bufs=1,
                                               side="right"))
        hTown = hop.tile([128, CCH, TOK], BF16)
        es_qkv = ExitStack()
        vp = es_qkv.enter_context(tc.tile_pool(name="Vp", bufs=1))
        V_sb = vp.tile([128, KB, H, 65], BF16)
        ktp = es_qkv.enter_context(tc.tile_pool(name="KTp", bufs=1))
        KT = ktp.tile([128, CCH, T], BF16)
        qtp = es_qkv.enter_context(tc.tile_pool(name="QTp", bufs=1))
        QT = qtp.tile([128, CCH, TOK], BF16)
        wqkp = es_qkv.enter_context(tc.tile_pool(name="wqkp", bufs=1))
        wq_full = wqkp.tile([128, CCH, C], BF16, tag="wq")
        nc.gpsimd.dma_start(wq_full[:], Wqv.transpose([1, 0, 2]))
        wk_full = wqkp.tile([128, CCH, C], BF16, tag="wk")
        nc.gpsimd.dma_start(wk_full[:], Wkv.transpose([1, 0, 2]))
        es_B = ExitStack()
        stB = es_B.enter_context(tc.tile_pool(name="stB", bufs=2))
        wv_gs = []
        for grp in range(2):
            wv_g = stB.tile([128, CCH, 512], BF16, tag="wv_g")
            nc.gpsimd.dma_start(
                wv_g[:],
                Wvv.transpose([1, 0, 2])[:, :, grp * 512:(grp + 1) * 512])
            wv_gs.append(wv_g)
        ones_b = const.tile([128, 1], BF16)
        nc.vector.memset(ones_b[:], 1.0)
        nc.vector.tensor_copy(
            V_sb[:, :, :, 64:65],
            ones_b[:, 0:1, None, None].to_broadcast([128, KB, H, 1]))

        # ==== Stage A' (LN1 of own rows -> hTown) interleaved with B3 (QT):
        # each 512-token group of hTown feeds its Q matmuls immediately =====
        with tc.tile_pool(name="stA2", bufs=3) as stA2:
            for nt in range(2):
                for tb in range(4 * nt, 4 * nt + 4):
                    x_t = stA2.tile([128, C], F32, tag="x_t")
                    nc.sync.dma_start(x_t[:], xo[tb * 128:(tb + 1) * 128, :])
                    mean, rstd = ln_stats(nc, stA2, x_t[:])
                    hrow = stA2.tile([128, C], BF16, tag="hrow")
                    ln_apply(nc, stA2, hrow[:], x_t[:], mean, rstd)
                    for cg in range(2):
                        pt = psT.tile([128, 512], BF16, tag="ptn")
                        for i in range(4):
                            cc = cg * 4 + i
                            nc.tensor.matmul(
                                pt[:, i * 128:(i + 1) * 128],
                                hrow[:, cc * 128:(cc + 1) * 128], ident_b[:],
                                is_transpose=True,
                                start=(i == 0), stop=(i == 3))
                        eng = (nc.scalar.copy if cg == 0
                               else nc.vector.tensor_copy)
                        eng(hTown[:, cg * 4:(cg + 1) * 4,
                                  tb * 128:(tb + 1) * 128],
                            pt[:].rearrange("p (g f) -> p g f", f=128))
                for pair in range(CCH):
                    pq = psK.tile([128, 512], F32, tag="pk")
                    for cc in range(CCH):
                        nc.tensor.matmul(
                            pq[:], wq_full[:, cc, pair * 128:(pair + 1) * 128],
                            hTown[:, cc, nt * 512:(nt + 1) * 512],
                            start=(cc == 0), stop=(cc == CCH - 1))
                    nc.scalar.activation(QT[:, pair, nt * 512:(nt + 1) * 512],
                                         pq[:], AF.Identity,
                                         bias=qb_sb[:, pair:pair + 1])
        es_ho.close()   # free hTown

        # ============ Stage A: LN1 over full T -> hT [128, CCH, T] bf16 ====
        with tc.tile_pool(name="stA", bufs=3) as stA:
            for tb in range(T // 128):
                x_t = stA.tile([128, C], F32, tag="x_t")
                nc.scalar.dma_start(x_t[:], xf[tb * 128:(tb + 1) * 128, :])
                mean, rstd = ln_stats(nc, stA, x_t[:])
                hrow = stA.tile([128, C], BF16, tag="hrow")
                ln_apply(nc, stA, hrow[:], x_t[:], mean, rstd)
                for cg in range(2):
                    pt = psT.tile([128, 512], BF16, tag="ptn")
                    for i in range(4):
                        cc = cg * 4 + i
                        nc.tensor.matmul(
                            pt[:, i * 128:(i + 1) * 128],
                            hrow[:, cc * 128:(cc + 1) * 128], ident_b[:],
                            is_transpose=True, start=(i == 0), stop=(i == 3))
                    eng = (nc.scalar.copy if cg == 0
                           else nc.vector.tensor_copy)
                    eng(hT[:, cg * 4:(cg + 1) * 4, tb * 128:(tb + 1) * 128],
                        pt[:].rearrange("p (g f) -> p g f", f=128))

        # ============ Stage B1: V (token-major, bf16, ones-augmented) ======
        for grp in range(2):
            wv_g = wv_gs[grp]
            for tb in range(KB):
                pv = psMM.tile([128, 512], F32, tag="mm")
                for cc in range(CCH):
                    nc.tensor.matmul(pv[:], hT[:, cc, tb * 128:(tb + 1) * 128],
                                     wv_g[:, cc], start=(cc == 0),
                                     stop=(cc == CCH - 1))
                eng = nc.scalar.copy if tb % 2 == 0 else nc.vector.tensor_copy
                eng(V_sb[:, tb, grp * 8:(grp + 1) * 8, 0:64],
                    pv[:].rearrange("p (h d) -> p h d", d=64))
        es_B.close()  # free wv_g

        # ============ Stages B2 + C interleaved: K(pair) then attention ====
        es_at = ExitStack()
        atp = es_at.enter_context(tc.tile_pool(name="attnT", bufs=1,
                                               side="right"))
        attnT_sb = atp.tile([128, CCH, TOK], BF16)
        wpp = ExitStack()
        wp_pool = wpp.enter_context(tc.tile_pool(name="wpf", bufs=1,
                                                 side="right"))
        wp_full = wp_pool.tile([128, CCH, C], BF16)
        nc.gpsimd.dma_start(wp_full[:], Wpv.transpose([1, 0, 2]))
        es_C = ExitStack()
        stC = es_C.enter_context(tc.tile_pool(name="stC", bufs=4))
        stC2 = es_C.enter_context(tc.tile_pool(name="stC2", bufs=3))
        av_gate = {}
        for pair in range(CCH):
            # ---- B2(pair): KT[:, pair, :] over full T ----
            # Gate B2(pair>=2) on attention progress of pair-2 so its matmuls
            # spread through the exp-paced attention tail instead of all
            # running up front.
            for nt in range(T // 512):
                pk = psK.tile([128, 512], F32, tag="pk")
                for cc in range(CCH):
                    mm = nc.tensor.matmul(
                        pk[:], wk_full[:, cc, pair * 128:(pair + 1) * 128],
                        hT[:, cc, nt * 512:(nt + 1) * 512],
                        start=(cc == 0), stop=(cc == CCH - 1))
                    if pair >= 2 and pair - 2 in av_gate:
                        add_dep_helper(mm.ins, av_gate[pair - 2].ins,
                                       sync=True, reason="spread B2 filler")
                # K bias on DVE (ACT is saturated by exp during this phase)
                nc.vector.tensor_scalar(KT[:, pair, nt * 512:(nt + 1) * 512],
                                        pk[:], kb_sb[:, pair:pair + 1], None,
                                        op0=ALU.add)
            # ---- C: the two heads of this pair ----
            for h in (2 * pair, 2 * pair + 1):
                off = 64 * (h % 2)
                ps_att = psW.tile([128, TOK], F32, tag="wide")
                for k in range(KB):
                    jmin = k // 2
                    q0 = jmin * 128
                    nq = TOK - q0
                    weiT = stC.tile([128, TOK], BF16, tag="weiT")
                    for qa in range(0, nq, 512):
                        qn = min(512, nq - qa)
                        if (k + qa // 512) % 2 == 0:
                            ps_s = psMM.tile([128, 512], F32, tag="mm")
                        else:
                            ps_s = psK.tile([128, 512], F32, tag="pk")
                        nc.tensor.matmul(
                            ps_s[:, 0:qn],
                            KT[off:off + 64, pair, k * 128:(k + 1) * 128],
                            QT[off:off + 64, pair, q0 + qa:q0 + qa + qn],
                            start=True, stop=True)
                        nc.scalar.activation(weiT[:, qa:qa + qn],
                                             ps_s[:, 0:qn], AF.Exp,
                                             scale=0.125)
                    nc.gpsimd.tensor_tensor(weiT[:, 0:128], weiT[:, 0:128],
                                            biasm[:, jmin, k - 2 * jmin],
                                            ALU.mult)
                    # AV: one matmul per 512-col PSUM bank
                    if k <= 7:  # bank 0: q cols [q0, 512)
                        nc.tensor.matmul(
                            ps_att[0:65, q0:512],
                            V_sb[:, k, h, :],
                            weiT[:, 0:512 - q0],
                            start=(k == 0), stop=(k == 7))
                    b1lo = max(512, q0)  # bank 1: q cols [b1lo, 1024)
                    av = nc.tensor.matmul(
                        ps_att[0:65, b1lo:TOK],
                        V_sb[:, k, h, :],
                        weiT[:, b1lo - q0:TOK - q0],
                        start=(k == 0), stop=(k == KB - 1))
                    if h % 2 == 1 and k == 8:
                        av_gate[pair] = av
                # normalize + transpose into attnT (SBUF). Row 64 of sb_at
                # gets the RECIPROCAL of the rowsum, so after the per-block
                # transpose the per-token 1/denom sits in column 64.
                sb_at = stC2.tile([128, TOK], BF16, tag="sb_at")
                # per-bank copies: bank 0 (cols 0:512) is final after k==7,
                # so it copies while bank 1 still accumulates
                nc.vector.tensor_copy(sb_at[0:65, 0:512],
                                      ps_att[0:65, 0:512])
                nc.vector.tensor_copy(sb_at[0:65, 512:TOK],
                                      ps_att[0:65, 512:TOK])
                for j in range(NB):
                    pt1 = psT.tile([128, 128], BF16, tag="ptn")
                    nc.tensor.transpose(pt1[:],
                                        sb_at[:, j * 128:(j + 1) * 128],
                                        ident_b[:])
                    recip = stC.tile([128, 1], F32, tag="recip")
                    nc.vector.reciprocal(recip[:], pt1[:, 64:65])
                    attn_j = stC.tile([128, 64], BF16, tag="attn_j")
                    nc.vector.tensor_scalar_mul(attn_j[:], pt1[:, 0:64],
                                                recip[:])
                    pt2 = psT.tile([128, 128], BF16, tag="ptn")
                    # transpose directly into partitions [off, off+64) —
                    # engines are lane-locked, so the copy below must be
                    # lane-aligned
                    nc.tensor.transpose(pt2[off:off + 64, :], attn_j[:],
                                        ident_b[:])
                    nc.vector.tensor_scalar(
                        attnT_sb[off:off + 64, pair, j * 128:(j + 1) * 128],
                        pt2[off:off + 64, :],
                        vb_sb[off:off + 64, pair:pair + 1],
                        None, op0=ALU.add)
        es_C.close()     # free weiT / sb_at working tiles
        es_qkv.close()   # free V, KT, QT

        # ============ Stage D: Wp proj (token-major) + residual + LN2 ======
        x2p = top.enter_context(tc.tile_pool(name="x2h2", bufs=1))
        x2_sb = x2p.tile([128, NB, C], F32)
        h2T = x2p.tile([128, CCH, TOK], BF16)
        stD = wpp.enter_context(tc.tile_pool(name="stD", bufs=4))
        for tb in range(NB):
            psx0 = psMM.tile([128, 512], F32, tag="mm")
            psx1 = psK.tile([128, 512], F32, tag="pk")
            psx = [psx0, psx1]
            for cc in range(CCH):
                for ci, c0 in enumerate((0, 512)):
                    nc.tensor.matmul(
                        psx[ci][:],
                        attnT_sb[:, cc, tb * 128:(tb + 1) * 128],
                        wp_full[:, cc, c0:c0 + 512],
                        start=(cc == 0), stop=False)
            for ci, c0 in enumerate((0, 512)):  # + bp (rank-1 ones row)
                nc.tensor.matmul(psx[ci][:], ones1[:],
                                 bp_sb[:, c0:c0 + 512], start=False, stop=True)
            xo_t = stD.tile([128, C], F32, tag="xo_t")
            nc.sync.dma_start(xo_t[:], xo[tb * 128:(tb + 1) * 128, :])
            for ci, c0 in enumerate((0, 512)):
                nc.vector.tensor_tensor(x2_sb[:, tb, c0:c0 + 512], psx[ci][:],
                                        xo_t[:, c0:c0 + 512], ALU.add)
            # LN2
            mean, rstd = ln_stats(nc, stD, x2_sb[:, tb, :])
            h2row = stD.tile([128, C], BF16, tag="h2row")
            ln_apply(nc, stD, h2row[:], x2_sb[:, tb, :], mean, rstd)
            for cg in range(2):
                pt = psT.tile([128, 512], BF16, tag="ptn")
                for i in range(4):
                    cc = cg * 4 + i
                    nc.tensor.matmul(
                        pt[:, i * 128:(i + 1) * 128],
                        h2row[:, cc * 128:(cc + 1) * 128], ident_b[:],
                        is_transpose=True, start=(i == 0), stop=(i == 3))
                eng = nc.scalar.copy if cg == 0 else nc.vector.tensor_copy
                eng(h2T[:, cg * 4:(cg + 1) * 4, tb * 128:(tb + 1) * 128],
                    pt[:].rearrange("p (g f) -> p g f", f=128))
        wpp.close()    # free Wp + stage-D working tiles
        es_at.close()  # free attnT
        es_h.close()   # free hT (kept below attnT for LIFO pool order)

        # ============ Stage E: MLP (ff-halves; W2 swapped, token-major) ====
        accp = top.enter_context(tc.tile_pool(name="accp", bufs=1))
        acc_sb = accp.tile([128, NB, C], BF16)
        stE = top.enter_context(tc.tile_pool(name="stE", bufs=2))
        stE2 = top.enter_context(tc.tile_pool(name="stE2", bufs=1))
        stF = top.enter_context(tc.tile_pool(name="stF", bufs=3))
        for fh in range(2):
            es_half = ExitStack()
            ffp = es_half.enter_context(tc.tile_pool(name="ff1T", bufs=1))
            ff1T = ffp.tile([128, FH, TOK], BF16)
            for fog in range(4):
                w1g = stE.tile([128, CCH, 512], BF16, tag="w1g")
                nc.gpsimd.dma_start(
                    w1g[:], W1v.transpose([1, 0, 2])
                    [:, :, fh * 2048 + fog * 512:fh * 2048 + (fog + 1) * 512])
                for f4 in range(4):
                    fo = fog * 4 + f4          # local ff chunk in this half
                    for nt in range(TOK // 512):
                        pf = psMM.tile([128, 512], F32, tag="mm")
                        for cc in range(CCH):
                            nc.tensor.matmul(
                                pf[:], w1g[:, cc, f4 * 128:(f4 + 1) * 128],
                                h2T[:, cc, nt * 512:(nt + 1) * 512],
                                start=(cc == 0), stop=(cc == CCH - 1))
                        nc.scalar.activation(
                            ff1T[:, fo, nt * 512:(nt + 1) * 512], pf[:],
                            AF.Relu,
                            bias=b1p[:, fh * FH + fo:fh * FH + fo + 1])
            w2h = stE2.tile([128, FH, C], BF16, tag="w2h")
            nc.gpsimd.dma_start(w2h[:], W2v[:, fh * FH:(fh + 1) * FH, :])
            for tb in range(NB):
                p2a = psMM.tile([128, 512], F32, tag="mm")
                p2b = psK.tile([128, 512], F32, tag="pk")
                p2 = [p2a, p2b]
                for fo in range(FH):
                    for ci, c0 in enumerate((0, 512)):
                        nc.tensor.matmul(
                            p2[ci][:],
                            ff1T[:, fo, tb * 128:(tb + 1) * 128],
                            w2h[:, fo, c0:c0 + 512],
                            start=(fo == 0),
                            stop=(fh == 1 and fo == FH - 1))
                if fh == 0:
                    for ci, c0 in enumerate((0, 512)):  # + b2 (rank-1)
                        nc.tensor.matmul(p2[ci][:], ones1[:],
                                         b2_sb[:, c0:c0 + 512],
                                         start=False, stop=True)
                        eng = (nc.scalar.copy if ci == 0
                               else nc.vector.tensor_copy)
                        eng(acc_sb[:, tb, c0:c0 + 512], p2[ci][:])
                else:
                    out_t = stF.tile([128, C], F32, tag="out_t")
                    for ci, c0 in enumerate((0, 512)):
                        nc.vector.tensor_tensor(out_t[:, c0:c0 + 512],
                                                p2[ci][:],
                                                x2_sb[:, tb, c0:c0 + 512],
                                                ALU.add)
                    nc.vector.tensor_tensor(out_t[:], out_t[:],
                                            acc_sb[:, tb, :], ALU.add)
                    nc.sync.dma_start(out_d.ap()[tb * 128:(tb + 1) * 128, :],
                                      out_t[:])
            es_half.close()


def shard_inputs(inputs):
    """Full inputs dict -> list of 8 per-core in_maps.

    Folds LN1 gain/bias into Wq/Wk/Wv (weights scaled by g1 per input channel,
    be1 contribution becomes an additive bias on q/k/v) and LN2's into W1/b1.
    Weights are cast to bf16.
    """
    import ml_dtypes
    bf16 = ml_dtypes.bfloat16
    x = np.asarray(inputs["x"], np.float32)
    assert x.shape == (B, T, C)
    f64 = np.float64
    Wq = np.asarray(inputs["Wq"], f64); Wk = np.asarray(inputs["Wk"], f64)
    Wv = np.asarray(inputs["Wv"], f64); Wp = np.asarray(inputs["Wp"], f64)
    W1 = np.asarray(inputs["W1"], f64); W2 = np.asarray(inputs["W2"], f64)
    g1 = np.asarray(inputs["g1"], f64); be1 = np.asarray(inputs["be1"], f64)
    g2 = np.asarray(inputs["g2"], f64); be2 = np.asarray(inputs["be2"], f64)
    b1 = np.asarray(inputs["b1"], f64)
    shared = {
        "Wq": (g1[:, None] * Wq).astype(bf16),
        "Wk": (g1[:, None] * Wk).astype(bf16),
        "Wv": (g1[:, None] * Wv).astype(bf16),
        "Wp": Wp.astype(bf16), "W2": W2.astype(bf16),
        "W1": (g2[:, None] * W1).astype(bf16),
        "qbias": (be1 @ Wq).astype(np.float32).reshape(NB, 128),
        "kbias": (be1 @ Wk).astype(np.float32).reshape(NB, 128),
        "vbias": (be1 @ Wv).astype(np.float32).reshape(1, C),
        "b1": (b1 + be2 @ W1).astype(np.float32).reshape(1, FF),
        "bp": np.asarray(inputs["bp"], np.float32).astype(bf16).reshape(1, C),
        "b2": np.asarray(inputs["b2"], np.float32).astype(bf16).reshape(1, C),
    }
    in_maps = []
    for c in range(N_CORES):
        b, par = c // 2, c % 2
        gblocks = [2 * j + par for j in range(NB)]
        rows = np.concatenate([x[b, g * 128:(g + 1) * 128, :] for g in gblocks], 0)
        qpos = np.stack([np.arange(g * 128, (g + 1) * 128, dtype=np.float32)
                         for g in gblocks], 0)
        m = {"xfull": np.ascontiguousarray(x[b]),
             "xown": np.ascontiguousarray(rows), "qpos": qpos}
        m.update(shared)
        in_maps.append(m)
    return in_maps


def unshard_outputs(results):
    """list of per-core {'out': [TOK, C]} -> [B, T, C]"""
    out = np.zeros((B, T, C), np.float32)
    for c in range(N_CORES):
        b, par = c // 2, c % 2
        r = np.asarray(results[c]["out"])
        for j in range(NB):
            g = 2 * j + par
            out[b, g * 128:(g + 1) * 128, :] = r[j * 128:(j + 1) * 128, :]
    return out


_NC_CACHE = {}


def _get_nc():
    if "nc" not in _NC_CACHE:
        nc = bacc.Bacc("TRN2", target_bir_lowering=False, debug=False,
                       num_devices=N_CORES)
        build(nc, reps=1)
        nc.compile()
        _NC_CACHE["nc"] = nc
    return _NC_CACHE["nc"]


def kernel(**inputs):
    from concourse.bass_utils import run_bass_kernel_spmd
    nc = _get_nc()
    in_maps = shard_inputs(inputs)
    res = run_bass_kernel_spmd(nc, in_maps, core_ids=list(range(N_CORES)))
    return unshard_outputs(res.results)
